# revision 6
# baseline (speedup 1.0000x reference)
"""DiscreteBipartiteFlow forward on 8 Trainium2 NeuronCores.

Math (forward pass only):
  masked = mask * inputs                      (mask = 1 at odd l, 0 at even l)
  h   = relu(masked.reshape(B, L*V) @ W1 + b1)
  net = (h @ W2 + b2).reshape(B, L, 2V)
  loc, scale = argmax one-hots of net[..., :V], net[..., V:]
  out[odd l]  = inputs
  out[even l] = onehot((inv(scale) * ((tok - loc) mod V)) mod V), or 0 if scale==0

st_one_hot_argmax's forward value is exactly the hard one-hot, so the
post-MLP flow is pure index arithmetic mod 23.

Sharding (8 cores):
  mm1: tensor-parallel over hidden; core k computes hT[512k:512k+512, :]
       from the odd-position one-hot rows (2944 of 5888 W1 rows).
       2 bf16 passes (W1 hi/lo; x one-hot is bf16-exact), fp32 PSUM,
       relu+bias via ScalarE activation rounding to fp32r output.
  all-gather: h as fp32r (4 B/elem), one chunk per local 128-row tile,
       pipelined; a zero-dep dummy collective at t=0 absorbs the ~60-80us
       ncfw setup latency.
  mm2: tensor-parallel over output columns (736 even-position cols/core),
       single sweep over 32 contraction tiles grouped by gather chunk.
       TWO passes, both 1 cycle/row:
         pass1: fp32r  h_r x W2_r     (11-bit-mantissa operands)
         pass2: bf16   h_hi x W2_lo   (W2_lo = W2 - W2_r; h_hi cast on the
                otherwise-idle VectorE)
       W2 is fully SBUF-resident: half preloaded under mm1, half loaded
       into the space freed by the mm1 operands during the pre-mm2 stall,
       so the gather chunks never contend with W2 streaming. b2 is folded
       in as a rank-1 matmul per accumulator at the START of the group.
       The last chunk group runs m-outer so each batch-tile's accumulators
       close early; its epilogue + flow overlap the remaining matmuls.
  epilogue: argmax via is_ge + max-of-(BIG-iota); modular flow per batch
       tile; inverse mod 23 via one-hot table lookup.
"""

import numpy as np
import ml_dtypes

B, L, V = 512, 256, 23
H = 4096
NCORES = 8
HS = H // NCORES          # 512  hidden shard
HM = HS // 128            # 4    local hidden tiles
PS = L // NCORES          # 32   positions per core
EP = PS // 2              # 16   even positions per core
CW = PS * 2 * V           # 1472 net columns per core (incl. unused odd)
CE = EP * 2 * V           # 736  even-position net columns (the used ones)
NCH = 2                   # column chunks for mm2
CC = CE // NCH            # 368  columns per chunk (8 even positions)
KT1 = (L // 2) * V // 128  # 23 contraction tiles for mm1
KT2 = H // 128            # 32 contraction tiles for mm2
MT = B // 128             # 4 batch tiles
NPRE = 16                 # mm2 tile visits with W2 preloaded under mm1

BIG = 64.0
MAGIC = 12582912.0        # 1.5 * 2^23: float32 round-to-int domain
BF16 = ml_dtypes.bfloat16

_cache = {}


def _trunc11(a):
    """Round fp32 to 11 explicit mantissa bits (RNE) == TRN2 fp32r."""
    a = np.asarray(a, np.float32)
    m, e = np.frexp(a)
    m2 = np.round(m * 4096.0) / 4096.0
    return np.ldexp(m2, e).astype(np.float32)


def _build():
    import concourse.mybir as mybir
    import concourse.tile as tile
    from concourse import bacc

    fp32 = mybir.dt.float32
    fp32r = mybir.dt.float32r
    bf16 = mybir.dt.bfloat16
    Alu = mybir.AluOpType
    Act = mybir.ActivationFunctionType

    nc = bacc.Bacc("TRN2", target_bir_lowering=False, debug=False,
                   num_devices=NCORES)

    # ---- per-core inputs ----
    xt = nc.dram_tensor("xt", [KT1, 128, B], bf16, kind="ExternalInput")
    w1h = nc.dram_tensor("w1h", [KT1, 128, HS], bf16, kind="ExternalInput")
    w1l = nc.dram_tensor("w1l", [KT1, 128, HS], bf16, kind="ExternalInput")
    b1s = nc.dram_tensor("b1s", [HM, 128], fp32, kind="ExternalInput")
    w2r = nc.dram_tensor("w2r", [KT2, 128, CE], fp32r, kind="ExternalInput")
    w2l = nc.dram_tensor("w2l", [KT2, 128, CE], bf16, kind="ExternalInput")
    b2row = nc.dram_tensor("b2row", [1, CE], fp32r, kind="ExternalInput")
    inpe = nc.dram_tensor("inpe", [MT, 128, EP * V], bf16, kind="ExternalInput")
    oute = nc.dram_tensor("oute", [MT, 128, EP * V], bf16,
                          kind="ExternalOutput")

    # ---- constants (baked into the NEFF) ----
    iota_np = np.arange(V, dtype=np.float32)[None, :].repeat(128, 0)
    inv_np = np.array([0] + [pow(a, -1, V) for a in range(1, V)],
                      dtype=np.float32)[None, :].repeat(128, 0).astype(BF16)
    c_iota = nc.inline_tensor(np.ascontiguousarray(iota_np), name="c_iota")
    c_iob = nc.inline_tensor(np.ascontiguousarray(iota_np.astype(BF16)),
                             name="c_iob")
    c_bi = nc.inline_tensor(np.ascontiguousarray((BIG - iota_np).astype(BF16)),
                            name="c_bi")
    c_inv = nc.inline_tensor(np.ascontiguousarray(inv_np), name="c_inv")
    c_one = nc.inline_tensor(np.ones((1, 128), np.float32), name="c_one")

    # mm2 visit order: contraction tiles grouped by gather chunk
    order = [(g, s) for g in range(HM) for s in range(NCORES)]

    with tile.TileContext(nc) as tc:
        with (
            tc.tile_pool(name="persist", bufs=1) as persist,
            tc.tile_pool(name="hth", bufs=8) as hthp,
            tc.tile_pool(name="hb", bufs=8) as hbp,
            tc.tile_pool(name="ep", bufs=2) as ep,
            tc.tile_pool(name="flow", bufs=1) as flow,
            tc.tile_pool(name="small", bufs=2) as small,
            tc.tile_pool(name="ps", bufs=1, space="PSUM") as ps,
            tc.tile_pool(name="dram", bufs=1, space="DRAM") as dram,
        ):
            # ---------- dummy collective: absorbs ncfw setup latency --------
            warm_in = dram.tile([1, 16], fp32, tag="warm_in")
            warm_out = dram.tile([NCORES, 16], fp32, tag="warm_out",
                                 addr_space="Shared")
            nc.gpsimd.collective_compute(
                "AllGather", Alu.bypass,
                replica_groups=[list(range(NCORES))],
                ins=[warm_in.opt()], outs=[warm_out.opt()],
            )

            # ---------- constants to SBUF ----------
            iota_t = persist.tile([128, V], fp32, tag="iota")
            nc.sync.dma_start(iota_t[:], c_iota[:])
            iob_t = persist.tile([128, V], bf16, tag="iob")
            nc.sync.dma_start(iob_t[:], c_iob[:])
            cbi_t = persist.tile([128, V], bf16, tag="cbi")
            nc.sync.dma_start(cbi_t[:], c_bi[:])
            inv_t = persist.tile([128, V], bf16, tag="inv")
            nc.sync.dma_start(inv_t[:], c_inv[:])
            one_t = persist.tile([1, 128], fp32r, tag="one")
            nc.sync.dma_start(one_t[:], c_one[:].bitcast(fp32r))
            b1_t = []
            for m in range(HM):
                t = persist.tile([128, 1], fp32, tag=f"b1{m}")
                nc.sync.dma_start(t[:], b1s[m].unsqueeze(1))
                b1_t.append(t)
            b2_t = persist.tile([1, CE], fp32r, tag="b2")
            nc.sync.dma_start(b2_t[:], b2row[:])

            # ---------- per-chunk collective buffers ----------
            ag_in = [dram.tile([128, B], fp32r, tag=f"ag_in{m}",
                               name=f"ag_in{m}") for m in range(HM)]
            ag_out = [dram.tile([NCORES, 128, B], fp32r, tag=f"ag_out{m}",
                                name=f"ag_out{m}", addr_space="Shared")
                      for m in range(HM)]

            # ---------- W2 preload for the first NPRE visits ----------------
            w2pre = {}
            hr_t = [None] * HM

            with tc.tile_pool(name="mm1ops", bufs=1) as mm1p:
                # mm1 operand loads first (they gate the critical path)
                xt_t, w1h_t, w1l_t = [], [], []
                for k in range(KT1):
                    t = mm1p.tile([128, B], bf16, tag=f"xt{k}", name=f"xt{k}")
                    nc.sync.dma_start(t[:], xt[k])
                    xt_t.append(t)
                for k in range(KT1):
                    th = mm1p.tile([128, HS], bf16, tag=f"w1h{k}",
                                   name=f"w1h{k}")
                    nc.sync.dma_start(th[:], w1h[k])
                    w1h_t.append(th)
                for k in range(KT1):
                    tl = mm1p.tile([128, HS], bf16, tag=f"w1l{k}",
                                   name=f"w1l{k}")
                    nc.sync.dma_start(tl[:], w1l[k])
                    w1l_t.append(tl)

                for oi in range(NPRE):
                    g, s = order[oi]
                    j = HM * s + g
                    tr = persist.tile([128, CE], fp32r, tag=f"w2pr{oi}",
                                      name=f"w2pr{oi}")
                    nc.sync.dma_start(tr[:], w2r[j])
                    tl = persist.tile([128, CE], bf16, tag=f"w2pl{oi}",
                                      name=f"w2pl{oi}")
                    nc.sync.dma_start(tl[:], w2l[j])
                    w2pre[oi] = (tr, tl)

                # ------- phase 1: mm1 -> relu -> fp32r round -> gather ------
                for m in range(HM):
                    acc = ps.tile([128, B], fp32, tag=f"p4_{m}_0",
                                  name=f"ps1_{m}")
                    for k in range(KT1):
                        nc.tensor.matmul(acc[:],
                                         w1h_t[k][:, m * 128:(m + 1) * 128],
                                         xt_t[k][:], start=(k == 0), stop=False)
                    for k in range(KT1):
                        nc.tensor.matmul(acc[:],
                                         w1l_t[k][:, m * 128:(m + 1) * 128],
                                         xt_t[k][:], start=False,
                                         stop=(k == KT1 - 1))
                    hr = persist.tile([128, B], fp32r, tag=f"hr{m}",
                                      name=f"hr{m}")
                    nc.scalar.activation(hr[:], acc[:], Act.Relu, bias=b1_t[m],
                                         scale=1.0)
                    hr_t[m] = hr
                    nc.sync.dma_start(ag_in[m][:], hr[:])
                    nc.gpsimd.collective_compute(
                        "AllGather", Alu.bypass,
                        replica_groups=[list(range(NCORES))],
                        ins=[ag_in[m].opt()], outs=[ag_out[m].opt()],
                    )

                # token index per batch tile (dep-free, runs during mm1)
                tok_t = persist.tile([128, MT, EP], fp32, tag="tok")
                for m in range(MT):
                    it = ep.tile([128, EP * V], bf16, tag="inpe")
                    nc.sync.dma_start(it[:], inpe[m])
                    tmp = ep.tile([128, EP, V], bf16, tag="tokmul")
                    nc.vector.tensor_tensor(
                        tmp[:], it[:].rearrange("p (e v) -> p e v", v=V),
                        iob_t[:].unsqueeze(1).broadcast_to([128, EP, V]),
                        Alu.mult)
                    nc.vector.tensor_reduce(tok_t[:, m], tmp[:],
                                            axis=mybir.AxisListType.X,
                                            op=Alu.add)

            # mm1 operand pool closed: its SBUF is reused for the second half
            # of W2, DMA'd during the pre-mm2 stall (no gather contention).
            with tc.tile_pool(name="w2late", bufs=1) as w2late:
                for oi in range(NPRE, KT2):
                    g, s = order[oi]
                    j = HM * s + g
                    # gate this load on an earlier gather chunk's completion
                    # so it never contends with the critical chunk-0/1 wire
                    # (g2's W2 loads during chunk 1, g3's during chunk 2)
                    gate = ag_out[0 if oi < 3 * NCORES else 1]
                    tr = w2late.tile([128, CE], fp32r, tag=f"w2qr{oi}",
                                     name=f"w2qr{oi}")
                    nc.sync.dma_start(tr[0:1, 0:1], gate[0, 0:1, 0:1])
                    nc.sync.dma_start(tr[:], w2r[j])
                    tl = w2late.tile([128, CE], bf16, tag=f"w2ql{oi}",
                                     name=f"w2ql{oi}")
                    nc.sync.dma_start(tl[0:1, 0:1],
                                      gate[:].bitcast(bf16)[0, 0:1, 0:1])
                    nc.sync.dma_start(tl[:], w2l[j])
                    w2pre[oi] = (tr, tl)

                idx_t = persist.tile([128, MT, EP, 2], fp32, tag="idx")

                # ---------- phase 2: mm2 ----------------------------------
                def mm2_epilogue(acc, nch, m):
                    # acc: [128, 368] = 8 even positions x (loc|scale) x 23
                    ng = acc.rearrange("p (i s v) -> p i s v", s=2, v=V)
                    gmax = ep.tile([128, 8, 2], fp32, tag="gmax", name="gmax")
                    nc.vector.tensor_reduce(gmax[:], ng,
                                            axis=mybir.AxisListType.X,
                                            op=Alu.max)
                    eq = ep.tile([128, 8, 2, V], bf16, tag="eq", name="eq")
                    nc.vector.tensor_tensor(
                        eq[:], ng,
                        gmax[:].unsqueeze(3).broadcast_to([128, 8, 2, V]),
                        Alu.is_ge)
                    mt = ep.tile([128, 8, 2, V], bf16, tag="mt", name="mt")
                    nc.vector.tensor_tensor(
                        mt[:], eq[:],
                        cbi_t[:].unsqueeze(1).unsqueeze(1).broadcast_to(
                            [128, 8, 2, V]), Alu.mult)
                    tmax = ep.tile([128, 8, 2], bf16, tag="tmax", name="tmax")
                    nc.vector.tensor_reduce(tmax[:], mt[:],
                                            axis=mybir.AxisListType.X,
                                            op=Alu.max)
                    nc.vector.tensor_scalar(
                        idx_t[:, m, nch * 8:(nch + 1) * 8, :],
                        tmax[:], -1.0, BIG, Alu.mult, Alu.add)

                def mod23(dst_tag, src):
                    d = small.tile([128, EP], fp32, tag=dst_tag + "_d",
                                   name=dst_tag + "_d")
                    nc.vector.tensor_scalar(d[:], src[:], 1.0 / 23.0, -0.49,
                                            Alu.mult, Alu.add)
                    q = small.tile([128, EP], fp32, tag=dst_tag + "_q",
                                   name=dst_tag + "_q")
                    nc.vector.tensor_scalar(q[:], d[:], MAGIC, MAGIC,
                                            Alu.add, Alu.subtract)
                    r = small.tile([128, EP], fp32, tag=dst_tag + "_r",
                                   name=dst_tag + "_r")
                    nc.vector.scalar_tensor_tensor(r[:], q[:], -23.0, src[:],
                                                   Alu.mult, Alu.add)
                    return r

                def flow_out(m):
                    """argmax indices -> modular flow -> one-hot -> DRAM."""
                    loc = idx_t[:, m, :, 0]
                    scl = idx_t[:, m, :, 1]
                    tok = tok_t[:, m]
                    u0 = small.tile([128, EP], fp32, tag="u0", name="u0")
                    nc.vector.scalar_tensor_tensor(u0[:], tok, 23.0, loc,
                                                   Alu.add, Alu.subtract)
                    geu = small.tile([128, EP], fp32, tag="geu", name="geu")
                    nc.vector.tensor_single_scalar(geu[:], u0[:], 23.0,
                                                   Alu.is_ge)
                    u = small.tile([128, EP], fp32, tag="u", name="u")
                    nc.vector.scalar_tensor_tensor(u[:], geu[:], -23.0, u0[:],
                                                   Alu.mult, Alu.add)
                    # inv = inv_table[scl] via one-hot dot
                    eqi = flow.tile([128, EP, V], bf16, tag="eqi", name="eqi")
                    nc.vector.tensor_tensor(
                        eqi[:],
                        iota_t[:].unsqueeze(1).broadcast_to([128, EP, V]),
                        scl.unsqueeze(2).broadcast_to([128, EP, V]),
                        Alu.is_equal)
                    isel = flow.tile([128, EP, V], bf16, tag="isel",
                                     name="isel")
                    nc.vector.tensor_tensor(
                        isel[:], eqi[:],
                        inv_t[:].unsqueeze(1).broadcast_to([128, EP, V]),
                        Alu.mult)
                    inv = small.tile([128, EP], fp32, tag="invv", name="invv")
                    nc.vector.tensor_reduce(inv[:], isel[:],
                                            axis=mybir.AxisListType.X,
                                            op=Alu.add)
                    wprod = small.tile([128, EP], fp32, tag="wprod",
                                       name="wprod")
                    nc.vector.tensor_tensor(wprod[:], inv[:], u[:], Alu.mult)
                    wm = mod23("wm", wprod)
                    live = small.tile([128, EP], fp32, tag="live", name="live")
                    nc.vector.tensor_single_scalar(live[:], inv[:], 0.5,
                                                   Alu.is_ge)
                    w1p = small.tile([128, EP], fp32, tag="w1p", name="w1p")
                    nc.vector.tensor_single_scalar(w1p[:], wm[:], 1.0, Alu.add)
                    w2p = small.tile([128, EP], fp32, tag="w2p", name="w2p")
                    nc.vector.tensor_tensor(w2p[:], w1p[:], live[:], Alu.mult)
                    wfin = small.tile([128, EP], fp32, tag="wfin", name="wfin")
                    nc.vector.tensor_single_scalar(wfin[:], w2p[:], -1.0,
                                                   Alu.add)
                    oh = flow.tile([128, EP, V], bf16, tag="oh", name="oh")
                    nc.vector.tensor_tensor(
                        oh[:],
                        iota_t[:].unsqueeze(1).broadcast_to([128, EP, V]),
                        wfin[:].unsqueeze(2).broadcast_to([128, EP, V]),
                        Alu.is_equal)
                    nc.sync.dma_start(oute[m],
                                      oh[:].rearrange("p e v -> p (e v)"))

                accs = {(m, c): ps.tile([128, B], fp32, tag=f"p4_{m}_{c}",
                                        name=f"p4_{m}_{c}")
                        for m in range(MT) for c in range(NCH)}

                # open every accumulation group with the rank-1 b2 update
                # (depends only on constants -> fills the pre-mm2 stall)
                for m in range(MT):
                    for c in range(NCH):
                        nc.tensor.matmul(accs[(m, c)][:, :CC], one_t[:],
                                         b2_t[:, c * CC:(c + 1) * CC],
                                         start=True, stop=False)

                def visit_tiles(oi):
                    g, s = order[oi]
                    j = HM * s + g
                    w2rt, w2lt = w2pre[oi]
                    ht = hthp.tile([128, B], fp32r, tag="hth", name=f"hth{j}")
                    nc.sync.dma_start(ht[:], ag_out[g][s])
                    hb = hbp.tile([128, B], bf16, tag="hb", name=f"hb{j}")
                    nc.vector.tensor_copy(hb[:], ht[:])
                    return ht, hb, w2rt, w2lt

                def tile_mms(m, ht, hb, w2rt, w2lt, last):
                    lr = ht[:, m * 128:(m + 1) * 128]
                    lb = hb[:, m * 128:(m + 1) * 128]
                    for c in range(NCH):
                        nc.tensor.matmul(accs[(m, c)][:, :CC], lr,
                                         w2rt[:, c * CC:(c + 1) * CC],
                                         start=False, stop=False)
                    for c in range(NCH):
                        nc.tensor.matmul(accs[(m, c)][:, :CC], lb,
                                         w2lt[:, c * CC:(c + 1) * CC],
                                         start=False, stop=last)

                # groups g=0..2: j outer, m inner
                for oi in range(3 * NCORES):
                    ht, hb, w2rt, w2lt = visit_tiles(oi)
                    for m in range(MT):
                        tile_mms(m, ht, hb, w2rt, w2lt, last=False)
                # group g=3: m outer, j inner -> acc(m) closes early; its
                # epilogue + flow overlap the remaining batch tiles' matmuls
                g3 = [visit_tiles(oi) for oi in range(3 * NCORES, 4 * NCORES)]
                for m in range(MT):
                    for vi, (ht, hb, w2rt, w2lt) in enumerate(g3):
                        tile_mms(m, ht, hb, w2rt, w2lt,
                                 last=(vi == NCORES - 1))
                    for c in range(NCH):
                        mm2_epilogue(accs[(m, c)][:, :CC], c, m)
                    flow_out(m)

    nc.compile()
    return nc


def _split_bf16(a):
    hi = a.astype(BF16)
    lo = (a - hi.astype(np.float32)).astype(BF16)
    return hi, lo


def kernel(inputs, mask, W1, b1, W2, b2):
    from concourse.bass_utils import run_bass_kernel_spmd

    if "nc" not in _cache:
        _cache["nc"] = _build()
    nc = _cache["nc"]

    inputs = np.asarray(inputs, np.float32)
    mask = np.asarray(mask, np.float32)
    W1 = np.asarray(W1, np.float32)
    b1 = np.asarray(b1, np.float32)
    W2 = np.asarray(W2, np.float32)
    b2 = np.asarray(b2, np.float32)

    masked = inputs * mask[None, :, :]                    # [B, L, V]
    x_odd = masked[:, 1::2, :].reshape(B, (L // 2) * V)   # [512, 2944]
    xt_np = np.ascontiguousarray(x_odd.T.reshape(KT1, 128, B)).astype(BF16)
    W1_odd = W1.reshape(L, V, H)[1::2].reshape((L // 2) * V, H)

    in_maps = []
    for k in range(NCORES):
        w1s = W1_odd[:, k * HS:(k + 1) * HS]
        w1hi, w1lo = _split_bf16(w1s)
        # odd-position net columns are multiplied by (1-mask)=0 downstream:
        # only the 736 even-position columns of this core's W2 slice matter
        w2sl = W2[:, k * CW:(k + 1) * CW].reshape(H, PS, 2 * V)[:, 0::2, :]
        w2sl = w2sl.reshape(H, CE)
        w2r_np = _trunc11(w2sl)
        w2l_np = (w2sl - w2r_np).astype(BF16)
        b2s = b2[k * CW:(k + 1) * CW].reshape(PS, 2 * V)[0::2].reshape(CE)
        cols = slice(32 * k, 32 * k + 32, 2)
        inpe_np = inputs[:, cols, :].reshape(MT, 128, EP * V)
        in_maps.append({
            "xt": xt_np,
            "w1h": np.ascontiguousarray(w1hi.reshape(KT1, 128, HS)),
            "w1l": np.ascontiguousarray(w1lo.reshape(KT1, 128, HS)),
            "b1s": np.ascontiguousarray(b1[k * HS:(k + 1) * HS].reshape(-1, 128)),
            "w2r": np.ascontiguousarray(w2r_np.reshape(KT2, 128, CE)),
            "w2l": np.ascontiguousarray(w2l_np.reshape(KT2, 128, CE)),
            "b2row": np.ascontiguousarray(_trunc11(b2s).reshape(1, CE)),
            "inpe": np.ascontiguousarray(inpe_np.astype(BF16)),
        })

    res = run_bass_kernel_spmd(nc, in_maps, core_ids=list(range(NCORES)))
    _cache["last_result"] = res

    out = np.empty((B, L, V), np.float32)
    out[:, 1::2, :] = masked[:, 1::2, :]
    for k in range(NCORES):
        oe = res.results[k]["oute"].astype(np.float32).reshape(MT, 128, EP, V)
        out[:, 32 * k:32 * k + 32:2, :] = oe.reshape(B, EP, V)
    return out


# revision 7
# speedup vs baseline: 1.1508x; 1.1508x over previous
"""DiscreteBipartiteFlow forward on 8 Trainium2 NeuronCores.

Math (forward pass only):
  masked = mask * inputs                      (mask = 1 at odd l, 0 at even l)
  h   = relu(masked.reshape(B, L*V) @ W1 + b1)
  net = (h @ W2 + b2).reshape(B, L, 2V)
  loc, scale = argmax one-hots of net[..., :V], net[..., V:]
  out[odd l]  = inputs
  out[even l] = onehot((inv(scale) * ((tok - loc) mod V)) mod V), or 0 if scale==0

st_one_hot_argmax's forward value is exactly the hard one-hot, so the
post-MLP flow is pure index arithmetic mod 23.

Sharding (8 cores):
  mm1: tensor-parallel over hidden; core k computes hT[512k:512k+512, :]
       from the odd-position one-hot rows (2944 of 5888 W1 rows).
       2 bf16 passes (W1 hi/lo; x one-hot is bf16-exact), fp32 PSUM,
       relu+bias via ScalarE activation rounding to fp32r output.
  all-gather: h as fp32r (4 B/elem), one chunk per local 128-row tile,
       pipelined; a zero-dep dummy collective at t=0 absorbs the ~60-80us
       ncfw setup latency.
  mm2: tensor-parallel over output columns (736 even-position cols/core),
       single sweep over 32 contraction tiles grouped by gather chunk.
       TWO passes, both 1 cycle/row:
         pass1: fp32r  h_r x W2_r     (11-bit-mantissa operands)
         pass2: bf16   h_hi x W2_lo   (W2_lo = W2 - W2_r; h_hi cast on the
                otherwise-idle VectorE)
       W2 is fully SBUF-resident: half preloaded under mm1, half loaded
       into the space freed by the mm1 operands during the pre-mm2 stall,
       so the gather chunks never contend with W2 streaming. b2 is folded
       in as a rank-1 matmul per accumulator at the START of the group.
       The last chunk group runs m-outer so each batch-tile's accumulators
       close early; its epilogue + flow overlap the remaining matmuls.
  epilogue: argmax via is_ge + max-of-(BIG-iota); modular flow per batch
       tile; inverse mod 23 via one-hot table lookup.
"""

import numpy as np
import ml_dtypes

B, L, V = 512, 256, 23
H = 4096
NCORES = 8
HS = H // NCORES          # 512  hidden shard
HM = HS // 128            # 4    local hidden tiles
PS = L // NCORES          # 32   positions per core
EP = PS // 2              # 16   even positions per core
CW = PS * 2 * V           # 1472 net columns per core (incl. unused odd)
CE = EP * 2 * V           # 736  even-position net columns (the used ones)
NCH = 2                   # column chunks for mm2
CC = CE // NCH            # 368  columns per chunk (8 even positions)
KT1 = (L // 2) * V // 128  # 23 contraction tiles for mm1
KT2 = H // 128            # 32 contraction tiles for mm2
MT = B // 128             # 4 batch tiles
NPRE = 16                 # mm2 tile visits with W2 preloaded under mm1

BIG = 64.0
MAGIC = 12582912.0        # 1.5 * 2^23: float32 round-to-int domain
BF16 = ml_dtypes.bfloat16

_cache = {}


def _trunc11(a):
    """Round fp32 to 11 explicit mantissa bits (RNE) == TRN2 fp32r."""
    a = np.asarray(a, np.float32)
    m, e = np.frexp(a)
    m2 = np.round(m * 4096.0) / 4096.0
    return np.ldexp(m2, e).astype(np.float32)


def _build():
    import concourse.mybir as mybir
    import concourse.tile as tile
    from concourse import bacc

    fp32 = mybir.dt.float32
    fp32r = mybir.dt.float32r
    bf16 = mybir.dt.bfloat16
    Alu = mybir.AluOpType
    Act = mybir.ActivationFunctionType

    nc = bacc.Bacc("TRN2", target_bir_lowering=False, debug=False,
                   num_devices=NCORES)

    # ---- per-core inputs ----
    xt = nc.dram_tensor("xt", [KT1, 128, B], bf16, kind="ExternalInput")
    w1h = nc.dram_tensor("w1h", [KT1, 128, HS], bf16, kind="ExternalInput")
    w1l = nc.dram_tensor("w1l", [KT1, 128, HS], bf16, kind="ExternalInput")
    b1s = nc.dram_tensor("b1s", [HM, 128], fp32, kind="ExternalInput")
    w2r = nc.dram_tensor("w2r", [KT2, 128, CE], fp32r, kind="ExternalInput")
    w2l = nc.dram_tensor("w2l", [KT2, 128, CE], bf16, kind="ExternalInput")
    b2row = nc.dram_tensor("b2row", [1, CE], fp32r, kind="ExternalInput")
    inpe = nc.dram_tensor("inpe", [MT, 128, EP * V], bf16, kind="ExternalInput")
    oute = nc.dram_tensor("oute", [MT, 128, EP * V], bf16,
                          kind="ExternalOutput")

    # ---- constants (baked into the NEFF) ----
    iota_np = np.arange(V, dtype=np.float32)[None, :].repeat(128, 0)
    inv_np = np.array([0] + [pow(a, -1, V) for a in range(1, V)],
                      dtype=np.float32)[None, :].repeat(128, 0).astype(BF16)
    c_iota = nc.inline_tensor(np.ascontiguousarray(iota_np), name="c_iota")
    c_iob = nc.inline_tensor(np.ascontiguousarray(iota_np.astype(BF16)),
                             name="c_iob")
    c_bi = nc.inline_tensor(np.ascontiguousarray((BIG - iota_np).astype(BF16)),
                            name="c_bi")
    c_inv = nc.inline_tensor(np.ascontiguousarray(inv_np), name="c_inv")
    c_one = nc.inline_tensor(np.ones((1, 128), np.float32), name="c_one")

    # mm2 visit order: contraction tiles grouped by gather chunk
    order = [(g, s) for g in range(HM) for s in range(NCORES)]

    with tile.TileContext(nc) as tc:
        with (
            tc.tile_pool(name="persist", bufs=1) as persist,
            tc.tile_pool(name="hth", bufs=8) as hthp,
            tc.tile_pool(name="hb", bufs=8) as hbp,
            tc.tile_pool(name="ep", bufs=2) as ep,
            tc.tile_pool(name="flow", bufs=1) as flow,
            tc.tile_pool(name="small", bufs=2) as small,
            tc.tile_pool(name="ps", bufs=1, space="PSUM") as ps,
            tc.tile_pool(name="dram", bufs=1, space="DRAM") as dram,
        ):
            # ---------- dummy collective: absorbs ncfw setup latency --------
            warm_in = dram.tile([1, 16], fp32, tag="warm_in")
            warm_out = dram.tile([NCORES, 16], fp32, tag="warm_out",
                                 addr_space="Shared")
            nc.gpsimd.collective_compute(
                "AllGather", Alu.bypass,
                replica_groups=[list(range(NCORES))],
                ins=[warm_in.opt()], outs=[warm_out.opt()],
            )

            # ---------- constants to SBUF ----------
            iota_t = persist.tile([128, V], fp32, tag="iota")
            nc.sync.dma_start(iota_t[:], c_iota[:])
            iob_t = persist.tile([128, V], bf16, tag="iob")
            nc.sync.dma_start(iob_t[:], c_iob[:])
            cbi_t = persist.tile([128, V], bf16, tag="cbi")
            nc.sync.dma_start(cbi_t[:], c_bi[:])
            inv_t = persist.tile([128, V], bf16, tag="inv")
            nc.sync.dma_start(inv_t[:], c_inv[:])
            one_t = persist.tile([1, 128], fp32r, tag="one")
            nc.sync.dma_start(one_t[:], c_one[:].bitcast(fp32r))
            b1_t = []
            for m in range(HM):
                t = persist.tile([128, 1], fp32, tag=f"b1{m}")
                nc.sync.dma_start(t[:], b1s[m].unsqueeze(1))
                b1_t.append(t)
            b2_t = persist.tile([1, CE], fp32r, tag="b2")
            nc.sync.dma_start(b2_t[:], b2row[:])

            # ---------- per-chunk collective buffers ----------
            ag_in = [dram.tile([128, B], fp32r, tag=f"ag_in{m}",
                               name=f"ag_in{m}") for m in range(HM)]
            ag_out = [dram.tile([NCORES, 128, B], fp32r, tag=f"ag_out{m}",
                                name=f"ag_out{m}", addr_space="Shared")
                      for m in range(HM)]

            # ---------- W2 preload for the first NPRE visits ----------------
            w2pre = {}
            hr_t = [None] * HM

            with tc.tile_pool(name="mm1ops", bufs=1) as mm1p:
                # mm1 operand loads first (they gate the critical path)
                xt_t, w1h_t, w1l_t = [], [], []
                for k in range(KT1):
                    t = mm1p.tile([128, B], bf16, tag=f"xt{k}", name=f"xt{k}")
                    nc.sync.dma_start(t[:], xt[k])
                    xt_t.append(t)
                for k in range(KT1):
                    th = mm1p.tile([128, HS], bf16, tag=f"w1h{k}",
                                   name=f"w1h{k}")
                    nc.sync.dma_start(th[:], w1h[k])
                    w1h_t.append(th)
                for k in range(KT1):
                    tl = mm1p.tile([128, HS], bf16, tag=f"w1l{k}",
                                   name=f"w1l{k}")
                    nc.sync.dma_start(tl[:], w1l[k])
                    w1l_t.append(tl)

                for oi in range(NPRE):
                    g, s = order[oi]
                    j = HM * s + g
                    tr = persist.tile([128, CE], fp32r, tag=f"w2pr{oi}",
                                      name=f"w2pr{oi}")
                    nc.sync.dma_start(tr[:], w2r[j])
                    tl = persist.tile([128, CE], bf16, tag=f"w2pl{oi}",
                                      name=f"w2pl{oi}")
                    nc.sync.dma_start(tl[:], w2l[j])
                    w2pre[oi] = (tr, tl)

                # ------- phase 1: mm1 -> relu -> fp32r round -> gather ------
                for m in range(HM):
                    acc = ps.tile([128, B], fp32, tag=f"p4_{m}_0",
                                  name=f"ps1_{m}")
                    for k in range(KT1):
                        nc.tensor.matmul(acc[:],
                                         w1h_t[k][:, m * 128:(m + 1) * 128],
                                         xt_t[k][:], start=(k == 0), stop=False)
                    for k in range(KT1):
                        nc.tensor.matmul(acc[:],
                                         w1l_t[k][:, m * 128:(m + 1) * 128],
                                         xt_t[k][:], start=False,
                                         stop=(k == KT1 - 1))
                    hr = persist.tile([128, B], fp32r, tag=f"hr{m}",
                                      name=f"hr{m}")
                    nc.scalar.activation(hr[:], acc[:], Act.Relu, bias=b1_t[m],
                                         scale=1.0)
                    hr_t[m] = hr
                    nc.sync.dma_start(ag_in[m][:], hr[:])
                    nc.gpsimd.collective_compute(
                        "AllGather", Alu.bypass,
                        replica_groups=[list(range(NCORES))],
                        ins=[ag_in[m].opt()], outs=[ag_out[m].opt()],
                    )

                # token index per batch tile (dep-free, runs during mm1)
                tok_t = persist.tile([128, MT, EP], fp32, tag="tok")
                for m in range(MT):
                    it = ep.tile([128, EP * V], bf16, tag="inpe")
                    nc.sync.dma_start(it[:], inpe[m])
                    tmp = ep.tile([128, EP, V], bf16, tag="tokmul")
                    nc.vector.tensor_tensor(
                        tmp[:], it[:].rearrange("p (e v) -> p e v", v=V),
                        iob_t[:].unsqueeze(1).broadcast_to([128, EP, V]),
                        Alu.mult)
                    nc.vector.tensor_reduce(tok_t[:, m], tmp[:],
                                            axis=mybir.AxisListType.X,
                                            op=Alu.add)

            # mm1 operand pool closed: its SBUF is reused for the second half
            # of W2, DMA'd during the pre-mm2 stall (no gather contention).
            # second half of W2 loads via a rotating pool: each load waits
            # (WAR) for the visit 8 slots earlier to finish reading, which
            # paces this traffic well past the critical chunk-0/1 windows.
            with tc.tile_pool(name="w2late", bufs=8) as w2late:
                idx_t = persist.tile([128, MT, EP, 2], fp32, tag="idx")

                # ---------- phase 2: mm2 ----------------------------------
                def mm2_epilogue(acc, nch, m):
                    # acc: [128, 368] = 8 even positions x (loc|scale) x 23
                    ng = acc.rearrange("p (i s v) -> p i s v", s=2, v=V)
                    gmax = ep.tile([128, 8, 2], fp32, tag="gmax", name="gmax")
                    nc.vector.tensor_reduce(gmax[:], ng,
                                            axis=mybir.AxisListType.X,
                                            op=Alu.max)
                    eq = ep.tile([128, 8, 2, V], bf16, tag="eq", name="eq")
                    nc.vector.tensor_tensor(
                        eq[:], ng,
                        gmax[:].unsqueeze(3).broadcast_to([128, 8, 2, V]),
                        Alu.is_ge)
                    mt = ep.tile([128, 8, 2, V], bf16, tag="mt", name="mt")
                    nc.vector.tensor_tensor(
                        mt[:], eq[:],
                        cbi_t[:].unsqueeze(1).unsqueeze(1).broadcast_to(
                            [128, 8, 2, V]), Alu.mult)
                    tmax = ep.tile([128, 8, 2], bf16, tag="tmax", name="tmax")
                    nc.vector.tensor_reduce(tmax[:], mt[:],
                                            axis=mybir.AxisListType.X,
                                            op=Alu.max)
                    nc.vector.tensor_scalar(
                        idx_t[:, m, nch * 8:(nch + 1) * 8, :],
                        tmax[:], -1.0, BIG, Alu.mult, Alu.add)

                def mod23(dst_tag, src):
                    d = small.tile([128, EP], fp32, tag=dst_tag + "_d",
                                   name=dst_tag + "_d")
                    nc.vector.tensor_scalar(d[:], src[:], 1.0 / 23.0, -0.49,
                                            Alu.mult, Alu.add)
                    q = small.tile([128, EP], fp32, tag=dst_tag + "_q",
                                   name=dst_tag + "_q")
                    nc.vector.tensor_scalar(q[:], d[:], MAGIC, MAGIC,
                                            Alu.add, Alu.subtract)
                    r = small.tile([128, EP], fp32, tag=dst_tag + "_r",
                                   name=dst_tag + "_r")
                    nc.vector.scalar_tensor_tensor(r[:], q[:], -23.0, src[:],
                                                   Alu.mult, Alu.add)
                    return r

                def flow_out(m):
                    """argmax indices -> modular flow -> one-hot -> DRAM."""
                    loc = idx_t[:, m, :, 0]
                    scl = idx_t[:, m, :, 1]
                    tok = tok_t[:, m]
                    u0 = small.tile([128, EP], fp32, tag="u0", name="u0")
                    nc.vector.scalar_tensor_tensor(u0[:], tok, 23.0, loc,
                                                   Alu.add, Alu.subtract)
                    geu = small.tile([128, EP], fp32, tag="geu", name="geu")
                    nc.vector.tensor_single_scalar(geu[:], u0[:], 23.0,
                                                   Alu.is_ge)
                    u = small.tile([128, EP], fp32, tag="u", name="u")
                    nc.vector.scalar_tensor_tensor(u[:], geu[:], -23.0, u0[:],
                                                   Alu.mult, Alu.add)
                    # inv = inv_table[scl] via one-hot dot
                    eqi = flow.tile([128, EP, V], bf16, tag="eqi", name="eqi")
                    nc.vector.tensor_tensor(
                        eqi[:],
                        iota_t[:].unsqueeze(1).broadcast_to([128, EP, V]),
                        scl.unsqueeze(2).broadcast_to([128, EP, V]),
                        Alu.is_equal)
                    isel = flow.tile([128, EP, V], bf16, tag="isel",
                                     name="isel")
                    nc.vector.tensor_tensor(
                        isel[:], eqi[:],
                        inv_t[:].unsqueeze(1).broadcast_to([128, EP, V]),
                        Alu.mult)
                    inv = small.tile([128, EP], fp32, tag="invv", name="invv")
                    nc.vector.tensor_reduce(inv[:], isel[:],
                                            axis=mybir.AxisListType.X,
                                            op=Alu.add)
                    wprod = small.tile([128, EP], fp32, tag="wprod",
                                       name="wprod")
                    nc.vector.tensor_tensor(wprod[:], inv[:], u[:], Alu.mult)
                    wm = mod23("wm", wprod)
                    live = small.tile([128, EP], fp32, tag="live", name="live")
                    nc.vector.tensor_single_scalar(live[:], inv[:], 0.5,
                                                   Alu.is_ge)
                    w1p = small.tile([128, EP], fp32, tag="w1p", name="w1p")
                    nc.vector.tensor_single_scalar(w1p[:], wm[:], 1.0, Alu.add)
                    w2p = small.tile([128, EP], fp32, tag="w2p", name="w2p")
                    nc.vector.tensor_tensor(w2p[:], w1p[:], live[:], Alu.mult)
                    wfin = small.tile([128, EP], fp32, tag="wfin", name="wfin")
                    nc.vector.tensor_single_scalar(wfin[:], w2p[:], -1.0,
                                                   Alu.add)
                    oh = flow.tile([128, EP, V], bf16, tag="oh", name="oh")
                    nc.vector.tensor_tensor(
                        oh[:],
                        iota_t[:].unsqueeze(1).broadcast_to([128, EP, V]),
                        wfin[:].unsqueeze(2).broadcast_to([128, EP, V]),
                        Alu.is_equal)
                    nc.sync.dma_start(oute[m],
                                      oh[:].rearrange("p e v -> p (e v)"))

                accs = {(m, c): ps.tile([128, B], fp32, tag=f"p4_{m}_{c}",
                                        name=f"p4_{m}_{c}")
                        for m in range(MT) for c in range(NCH)}

                # open every accumulation group with the rank-1 b2 update
                # (depends only on constants -> fills the pre-mm2 stall)
                for m in range(MT):
                    for c in range(NCH):
                        nc.tensor.matmul(accs[(m, c)][:, :CC], one_t[:],
                                         b2_t[:, c * CC:(c + 1) * CC],
                                         start=True, stop=False)

                def visit_tiles(oi):
                    g, s = order[oi]
                    j = HM * s + g
                    if oi < NPRE:
                        w2rt, w2lt = w2pre[oi]
                    else:
                        w2rt = w2late.tile([128, CE], fp32r, tag="w2qr",
                                           name=f"w2qr{oi}")
                        nc.sync.dma_start(w2rt[:], w2r[j])
                        w2lt = w2late.tile([128, CE], bf16, tag="w2ql",
                                           name=f"w2ql{oi}")
                        nc.sync.dma_start(w2lt[:], w2l[j])
                    ht = hthp.tile([128, B], fp32r, tag="hth", name=f"hth{j}")
                    nc.sync.dma_start(ht[:], ag_out[g][s])
                    hb = hbp.tile([128, B], bf16, tag="hb", name=f"hb{j}")
                    nc.vector.tensor_copy(hb[:], ht[:])
                    return ht, hb, w2rt, w2lt

                def tile_mms(m, ht, hb, w2rt, w2lt, last):
                    lr = ht[:, m * 128:(m + 1) * 128]
                    lb = hb[:, m * 128:(m + 1) * 128]
                    for c in range(NCH):
                        nc.tensor.matmul(accs[(m, c)][:, :CC], lr,
                                         w2rt[:, c * CC:(c + 1) * CC],
                                         start=False, stop=False)
                    for c in range(NCH):
                        nc.tensor.matmul(accs[(m, c)][:, :CC], lb,
                                         w2lt[:, c * CC:(c + 1) * CC],
                                         start=False, stop=last)

                # groups g=0..2: j outer, m inner
                for oi in range(3 * NCORES):
                    ht, hb, w2rt, w2lt = visit_tiles(oi)
                    for m in range(MT):
                        tile_mms(m, ht, hb, w2rt, w2lt, last=False)
                # group g=3: m outer, j inner -> acc(m) closes early; its
                # epilogue + flow overlap the remaining batch tiles' matmuls
                g3 = [visit_tiles(oi) for oi in range(3 * NCORES, 4 * NCORES)]
                for m in range(MT):
                    for vi, (ht, hb, w2rt, w2lt) in enumerate(g3):
                        tile_mms(m, ht, hb, w2rt, w2lt,
                                 last=(vi == NCORES - 1))
                    for c in range(NCH):
                        mm2_epilogue(accs[(m, c)][:, :CC], c, m)
                    flow_out(m)

    nc.compile()
    return nc


def _split_bf16(a):
    hi = a.astype(BF16)
    lo = (a - hi.astype(np.float32)).astype(BF16)
    return hi, lo


def kernel(inputs, mask, W1, b1, W2, b2):
    from concourse.bass_utils import run_bass_kernel_spmd

    if "nc" not in _cache:
        _cache["nc"] = _build()
    nc = _cache["nc"]

    inputs = np.asarray(inputs, np.float32)
    mask = np.asarray(mask, np.float32)
    W1 = np.asarray(W1, np.float32)
    b1 = np.asarray(b1, np.float32)
    W2 = np.asarray(W2, np.float32)
    b2 = np.asarray(b2, np.float32)

    masked = inputs * mask[None, :, :]                    # [B, L, V]
    x_odd = masked[:, 1::2, :].reshape(B, (L // 2) * V)   # [512, 2944]
    xt_np = np.ascontiguousarray(x_odd.T.reshape(KT1, 128, B)).astype(BF16)
    W1_odd = W1.reshape(L, V, H)[1::2].reshape((L // 2) * V, H)

    in_maps = []
    for k in range(NCORES):
        w1s = W1_odd[:, k * HS:(k + 1) * HS]
        w1hi, w1lo = _split_bf16(w1s)
        # odd-position net columns are multiplied by (1-mask)=0 downstream:
        # only the 736 even-position columns of this core's W2 slice matter
        w2sl = W2[:, k * CW:(k + 1) * CW].reshape(H, PS, 2 * V)[:, 0::2, :]
        w2sl = w2sl.reshape(H, CE)
        w2r_np = _trunc11(w2sl)
        w2l_np = (w2sl - w2r_np).astype(BF16)
        b2s = b2[k * CW:(k + 1) * CW].reshape(PS, 2 * V)[0::2].reshape(CE)
        cols = slice(32 * k, 32 * k + 32, 2)
        inpe_np = inputs[:, cols, :].reshape(MT, 128, EP * V)
        in_maps.append({
            "xt": xt_np,
            "w1h": np.ascontiguousarray(w1hi.reshape(KT1, 128, HS)),
            "w1l": np.ascontiguousarray(w1lo.reshape(KT1, 128, HS)),
            "b1s": np.ascontiguousarray(b1[k * HS:(k + 1) * HS].reshape(-1, 128)),
            "w2r": np.ascontiguousarray(w2r_np.reshape(KT2, 128, CE)),
            "w2l": np.ascontiguousarray(w2l_np.reshape(KT2, 128, CE)),
            "b2row": np.ascontiguousarray(_trunc11(b2s).reshape(1, CE)),
            "inpe": np.ascontiguousarray(inpe_np.astype(BF16)),
        })

    res = run_bass_kernel_spmd(nc, in_maps, core_ids=list(range(NCORES)))
    _cache["last_result"] = res

    out = np.empty((B, L, V), np.float32)
    out[:, 1::2, :] = masked[:, 1::2, :]
    for k in range(NCORES):
        oe = res.results[k]["oute"].astype(np.float32).reshape(MT, 128, EP, V)
        out[:, 32 * k:32 * k + 32:2, :] = oe.reshape(B, EP, V)
    return out


# revision 8
# speedup vs baseline: 1.1968x; 1.0399x over previous
"""DiscreteBipartiteFlow forward on 8 Trainium2 NeuronCores.

Math (forward pass only):
  masked = mask * inputs                      (mask = 1 at odd l, 0 at even l)
  h   = relu(masked.reshape(B, L*V) @ W1 + b1)
  net = (h @ W2 + b2).reshape(B, L, 2V)
  loc, scale = argmax one-hots of net[..., :V], net[..., V:]
  out[odd l]  = inputs
  out[even l] = onehot((inv(scale) * ((tok - loc) mod V)) mod V), or 0 if scale==0

st_one_hot_argmax's forward value is exactly the hard one-hot, so the
post-MLP flow is pure index arithmetic mod 23.

Sharding (8 cores):
  mm1: tensor-parallel over hidden; core k computes hT[512k:512k+512, :]
       from the odd-position one-hot rows (2944 of 5888 W1 rows).
       2 bf16 passes (W1 hi/lo; x one-hot is bf16-exact), fp32 PSUM,
       relu+bias via ScalarE activation rounding to fp32r output.
  all-gather: h as fp32r (4 B/elem), one chunk per local 128-row tile,
       pipelined; a zero-dep dummy collective at t=0 absorbs the ~60-80us
       ncfw setup latency.
  mm2: tensor-parallel over output columns (736 even-position cols/core),
       single sweep over 32 contraction tiles grouped by gather chunk.
       TWO passes, both 1 cycle/row:
         pass1: fp32r  h_r x W2_r     (11-bit-mantissa operands)
         pass2: bf16   h_hi x W2_lo   (W2_lo = W2 - W2_r; h_hi cast on the
                otherwise-idle VectorE)
       W2 is fully SBUF-resident: half preloaded under mm1, half loaded
       into the space freed by the mm1 operands during the pre-mm2 stall,
       so the gather chunks never contend with W2 streaming. b2 is folded
       in as a rank-1 matmul per accumulator at the START of the group.
       The last chunk group runs m-outer so each batch-tile's accumulators
       close early; its epilogue + flow overlap the remaining matmuls.
  epilogue: argmax via is_ge + max-of-(BIG-iota); modular flow per batch
       tile; inverse mod 23 via one-hot table lookup.
"""

import numpy as np
import ml_dtypes

B, L, V = 512, 256, 23
H = 4096
NCORES = 8
HS = H // NCORES          # 512  hidden shard
HM = HS // 128            # 4    local hidden tiles
PS = L // NCORES          # 32   positions per core
EP = PS // 2              # 16   even positions per core
CW = PS * 2 * V           # 1472 net columns per core (incl. unused odd)
CE = EP * 2 * V           # 736  even-position net columns (the used ones)
NCH = 2                   # column chunks for mm2
CC = CE // NCH            # 368  columns per chunk (8 even positions)
KT1 = (L // 2) * V // 128  # 23 contraction tiles for mm1
KT2 = H // 128            # 32 contraction tiles for mm2
MT = B // 128             # 4 batch tiles
NPRE = 16                 # mm2 tile visits with W2 preloaded under mm1

BIG = 64.0
MAGIC = 12582912.0        # 1.5 * 2^23: float32 round-to-int domain
BF16 = ml_dtypes.bfloat16

_cache = {}


def _trunc11(a):
    """Round fp32 to 11 explicit mantissa bits (RNE) == TRN2 fp32r."""
    a = np.asarray(a, np.float32)
    m, e = np.frexp(a)
    m2 = np.round(m * 4096.0) / 4096.0
    return np.ldexp(m2, e).astype(np.float32)


def _build():
    import concourse.mybir as mybir
    import concourse.tile as tile
    from concourse import bacc

    fp32 = mybir.dt.float32
    fp32r = mybir.dt.float32r
    bf16 = mybir.dt.bfloat16
    Alu = mybir.AluOpType
    Act = mybir.ActivationFunctionType

    nc = bacc.Bacc("TRN2", target_bir_lowering=False, debug=False,
                   num_devices=NCORES)

    # ---- per-core inputs ----
    xt = nc.dram_tensor("xt", [128, KT1, B], bf16, kind="ExternalInput")
    w1h = nc.dram_tensor("w1h", [128, KT1, HS], bf16, kind="ExternalInput")
    w1l = nc.dram_tensor("w1l", [128, KT1, HS], bf16, kind="ExternalInput")
    b1s = nc.dram_tensor("b1s", [HM, 128], fp32, kind="ExternalInput")
    # w2r/w2l are laid out in mm2 VISIT order on the host: [128, visit, CE]
    w2r = nc.dram_tensor("w2r", [128, KT2, CE], fp32r, kind="ExternalInput")
    w2l = nc.dram_tensor("w2l", [128, KT2, CE], bf16, kind="ExternalInput")
    b2row = nc.dram_tensor("b2row", [1, CE], fp32r, kind="ExternalInput")
    inpe = nc.dram_tensor("inpe", [MT, 128, EP * V], bf16, kind="ExternalInput")
    oute = nc.dram_tensor("oute", [MT, 128, EP * V], bf16,
                          kind="ExternalOutput")

    # ---- constants (baked into the NEFF) ----
    iota_np = np.arange(V, dtype=np.float32)[None, :].repeat(128, 0)
    inv_np = np.array([0] + [pow(a, -1, V) for a in range(1, V)],
                      dtype=np.float32)[None, :].repeat(128, 0).astype(BF16)
    c_iota = nc.inline_tensor(np.ascontiguousarray(iota_np), name="c_iota")
    c_iob = nc.inline_tensor(np.ascontiguousarray(iota_np.astype(BF16)),
                             name="c_iob")
    c_bi = nc.inline_tensor(np.ascontiguousarray((BIG - iota_np).astype(BF16)),
                            name="c_bi")
    c_inv = nc.inline_tensor(np.ascontiguousarray(inv_np), name="c_inv")
    c_one = nc.inline_tensor(np.ones((1, 128), np.float32), name="c_one")

    # mm2 visit order: contraction tiles grouped by gather chunk
    order = [(g, s) for g in range(HM) for s in range(NCORES)]

    with tile.TileContext(nc) as tc:
        with (
            tc.tile_pool(name="persist", bufs=1) as persist,
            tc.tile_pool(name="hth", bufs=8) as hthp,
            tc.tile_pool(name="hb", bufs=8) as hbp,
            tc.tile_pool(name="ep", bufs=2) as ep,
            tc.tile_pool(name="flow", bufs=1) as flow,
            tc.tile_pool(name="small", bufs=2) as small,
            tc.tile_pool(name="ps", bufs=1, space="PSUM") as ps,
            tc.tile_pool(name="dram", bufs=1, space="DRAM") as dram,
        ):
            # ---------- dummy collective: absorbs ncfw setup latency --------
            warm_in = dram.tile([1, 16], fp32, tag="warm_in")
            warm_out = dram.tile([NCORES, 16], fp32, tag="warm_out",
                                 addr_space="Shared")
            nc.gpsimd.collective_compute(
                "AllGather", Alu.bypass,
                replica_groups=[list(range(NCORES))],
                ins=[warm_in.opt()], outs=[warm_out.opt()],
            )

            # ---------- constants to SBUF ----------
            iota_t = persist.tile([128, V], fp32, tag="iota")
            nc.sync.dma_start(iota_t[:], c_iota[:])
            iob_t = persist.tile([128, V], bf16, tag="iob")
            nc.sync.dma_start(iob_t[:], c_iob[:])
            cbi_t = persist.tile([128, V], bf16, tag="cbi")
            nc.sync.dma_start(cbi_t[:], c_bi[:])
            inv_t = persist.tile([128, V], bf16, tag="inv")
            nc.sync.dma_start(inv_t[:], c_inv[:])
            one_t = persist.tile([1, 128], fp32r, tag="one")
            nc.sync.dma_start(one_t[:], c_one[:].bitcast(fp32r))
            b1_t = []
            for m in range(HM):
                t = persist.tile([128, 1], fp32, tag=f"b1{m}")
                nc.sync.dma_start(t[:], b1s[m].unsqueeze(1))
                b1_t.append(t)
            b2_t = persist.tile([1, CE], fp32r, tag="b2")
            nc.sync.dma_start(b2_t[:], b2row[:])

            # ---------- per-chunk collective buffers ----------
            ag_in = [dram.tile([128, B], fp32r, tag=f"ag_in{m}",
                               name=f"ag_in{m}") for m in range(HM)]
            ag_out = [dram.tile([NCORES, 128, B], fp32r, tag=f"ag_out{m}",
                                name=f"ag_out{m}", addr_space="Shared")
                      for m in range(HM)]

            # ---------- W2 preload for the first NPRE visits ----------------
            w2pre = {}
            hr_t = [None] * HM

            with tc.tile_pool(name="mm1ops", bufs=1) as mm1p:
                # mm1 operand loads in a few big descriptors, chunked so the
                # first matmuls can start before the whole stream lands
                # (per-descriptor sync issue costs ~0.5us -- fewer is faster)
                xtt = mm1p.tile([128, KT1, B], bf16, tag="xtt", name="xtt")
                w1ht = mm1p.tile([128, KT1, HS], bf16, tag="w1ht", name="w1ht")
                w1lt = mm1p.tile([128, KT1, HS], bf16, tag="w1lt", name="w1lt")
                kr = [(0, 6), (6, 12), (12, 18), (18, KT1)]
                for k0, k1 in kr:
                    nc.sync.dma_start(xtt[:, k0:k1, :], xt[:, k0:k1, :])
                    nc.sync.dma_start(w1ht[:, k0:k1, :], w1h[:, k0:k1, :])
                for k0, k1 in [(0, 12), (12, KT1)]:
                    nc.sync.dma_start(w1lt[:, k0:k1, :], w1l[:, k0:k1, :])
                xt_t = [xtt[:, k, :] for k in range(KT1)]
                w1h_t = [w1ht[:, k, :] for k in range(KT1)]
                w1l_t = [w1lt[:, k, :] for k in range(KT1)]

                w2preR = persist.tile([128, NPRE, CE], fp32r, tag="w2preR",
                                      name="w2preR")
                nc.sync.dma_start(w2preR[:], w2r[:, 0:NPRE, :])
                w2preL = persist.tile([128, NPRE, CE], bf16, tag="w2preL",
                                      name="w2preL")
                nc.sync.dma_start(w2preL[:], w2l[:, 0:NPRE, :])
                for oi in range(NPRE):
                    w2pre[oi] = (w2preR[:, oi, :], w2preL[:, oi, :])

                # ------- phase 1: mm1 -> relu -> fp32r round -> gather ------
                for m in range(HM):
                    acc = ps.tile([128, B], fp32, tag=f"p4_{m}_0",
                                  name=f"ps1_{m}")
                    for k in range(KT1):
                        nc.tensor.matmul(acc[:],
                                         w1h_t[k][:, m * 128:(m + 1) * 128],
                                         xt_t[k], start=(k == 0), stop=False)
                    for k in range(KT1):
                        nc.tensor.matmul(acc[:],
                                         w1l_t[k][:, m * 128:(m + 1) * 128],
                                         xt_t[k], start=False,
                                         stop=(k == KT1 - 1))
                    hr = persist.tile([128, B], fp32r, tag=f"hr{m}",
                                      name=f"hr{m}")
                    nc.scalar.activation(hr[:], acc[:], Act.Relu, bias=b1_t[m],
                                         scale=1.0)
                    hr_t[m] = hr
                    nc.sync.dma_start(ag_in[m][:], hr[:])
                    nc.gpsimd.collective_compute(
                        "AllGather", Alu.bypass,
                        replica_groups=[list(range(NCORES))],
                        ins=[ag_in[m].opt()], outs=[ag_out[m].opt()],
                    )

                # token index per batch tile (dep-free, runs during mm1)
                tok_t = persist.tile([128, MT, EP], fp32, tag="tok")
                for m in range(MT):
                    it = ep.tile([128, EP * V], bf16, tag="inpe")
                    nc.sync.dma_start(it[:], inpe[m])
                    tmp = ep.tile([128, EP, V], bf16, tag="tokmul")
                    nc.vector.tensor_tensor(
                        tmp[:], it[:].rearrange("p (e v) -> p e v", v=V),
                        iob_t[:].unsqueeze(1).broadcast_to([128, EP, V]),
                        Alu.mult)
                    nc.vector.tensor_reduce(tok_t[:, m], tmp[:],
                                            axis=mybir.AxisListType.X,
                                            op=Alu.add)

            # mm1 operand pool closed: its SBUF is reused for the second half
            # of W2, DMA'd during the pre-mm2 stall (no gather contention).
            # second half of W2 loads via a rotating pool: each load waits
            # (WAR) for the allocation 8 slots earlier to be fully read. A
            # pre-generation of gate tiles (tiny DMA from the chunk-0 gather
            # output, then a tiny vector read) makes the first 8 real loads
            # wait for chunk 0, keeping its gather wire contention-free.
            with tc.tile_pool(name="w2late", bufs=8) as w2late:
                for i in range(8):
                    gr = w2late.tile([128, CE], fp32r, tag="w2qr",
                                     name=f"gr{i}")
                    nc.sync.dma_start(gr[0:1, 0:1], ag_out[0][0, 0:1, 0:1])
                    sr = small.tile([1, 1], fp32, tag="gsr", name="gsr")
                    nc.vector.tensor_copy(sr[:], gr[0:1, 0:1].bitcast(fp32))
                    gl = w2late.tile([128, CE], bf16, tag="w2ql",
                                     name=f"gl{i}")
                    nc.sync.dma_start(
                        gl[0:1, 0:1],
                        ag_out[0][:].bitcast(bf16)[0, 0:1, 0:1])
                    sl = small.tile([1, 1], fp32, tag="gsl", name="gsl")
                    nc.vector.tensor_copy(sl[:], gl[0:1, 0:1])
                idx_t = persist.tile([128, MT, EP, 2], fp32, tag="idx")

                # ---------- phase 2: mm2 ----------------------------------
                def mm2_epilogue(acc, nch, m):
                    # acc: [128, 368] = 8 even positions x (loc|scale) x 23
                    ng = acc.rearrange("p (i s v) -> p i s v", s=2, v=V)
                    gmax = ep.tile([128, 8, 2], fp32, tag="gmax", name="gmax")
                    nc.vector.tensor_reduce(gmax[:], ng,
                                            axis=mybir.AxisListType.X,
                                            op=Alu.max)
                    eq = ep.tile([128, 8, 2, V], bf16, tag="eq", name="eq")
                    nc.vector.tensor_tensor(
                        eq[:], ng,
                        gmax[:].unsqueeze(3).broadcast_to([128, 8, 2, V]),
                        Alu.is_ge)
                    mt = ep.tile([128, 8, 2, V], bf16, tag="mt", name="mt")
                    nc.vector.tensor_tensor(
                        mt[:], eq[:],
                        cbi_t[:].unsqueeze(1).unsqueeze(1).broadcast_to(
                            [128, 8, 2, V]), Alu.mult)
                    tmax = ep.tile([128, 8, 2], bf16, tag="tmax", name="tmax")
                    nc.vector.tensor_reduce(tmax[:], mt[:],
                                            axis=mybir.AxisListType.X,
                                            op=Alu.max)
                    nc.vector.tensor_scalar(
                        idx_t[:, m, nch * 8:(nch + 1) * 8, :],
                        tmax[:], -1.0, BIG, Alu.mult, Alu.add)

                def mod23(dst_tag, src):
                    d = small.tile([128, EP], fp32, tag=dst_tag + "_d",
                                   name=dst_tag + "_d")
                    nc.vector.tensor_scalar(d[:], src[:], 1.0 / 23.0, -0.49,
                                            Alu.mult, Alu.add)
                    q = small.tile([128, EP], fp32, tag=dst_tag + "_q",
                                   name=dst_tag + "_q")
                    nc.vector.tensor_scalar(q[:], d[:], MAGIC, MAGIC,
                                            Alu.add, Alu.subtract)
                    r = small.tile([128, EP], fp32, tag=dst_tag + "_r",
                                   name=dst_tag + "_r")
                    nc.vector.scalar_tensor_tensor(r[:], q[:], -23.0, src[:],
                                                   Alu.mult, Alu.add)
                    return r

                def flow_out(m):
                    """argmax indices -> modular flow -> one-hot -> DRAM."""
                    loc = idx_t[:, m, :, 0]
                    scl = idx_t[:, m, :, 1]
                    tok = tok_t[:, m]
                    u0 = small.tile([128, EP], fp32, tag="u0", name="u0")
                    nc.vector.scalar_tensor_tensor(u0[:], tok, 23.0, loc,
                                                   Alu.add, Alu.subtract)
                    geu = small.tile([128, EP], fp32, tag="geu", name="geu")
                    nc.vector.tensor_single_scalar(geu[:], u0[:], 23.0,
                                                   Alu.is_ge)
                    u = small.tile([128, EP], fp32, tag="u", name="u")
                    nc.vector.scalar_tensor_tensor(u[:], geu[:], -23.0, u0[:],
                                                   Alu.mult, Alu.add)
                    # inv = inv_table[scl] via one-hot dot
                    eqi = flow.tile([128, EP, V], bf16, tag="eqi", name="eqi")
                    nc.vector.tensor_tensor(
                        eqi[:],
                        iota_t[:].unsqueeze(1).broadcast_to([128, EP, V]),
                        scl.unsqueeze(2).broadcast_to([128, EP, V]),
                        Alu.is_equal)
                    isel = flow.tile([128, EP, V], bf16, tag="isel",
                                     name="isel")
                    nc.vector.tensor_tensor(
                        isel[:], eqi[:],
                        inv_t[:].unsqueeze(1).broadcast_to([128, EP, V]),
                        Alu.mult)
                    inv = small.tile([128, EP], fp32, tag="invv", name="invv")
                    nc.vector.tensor_reduce(inv[:], isel[:],
                                            axis=mybir.AxisListType.X,
                                            op=Alu.add)
                    wprod = small.tile([128, EP], fp32, tag="wprod",
                                       name="wprod")
                    nc.vector.tensor_tensor(wprod[:], inv[:], u[:], Alu.mult)
                    wm = mod23("wm", wprod)
                    live = small.tile([128, EP], fp32, tag="live", name="live")
                    nc.vector.tensor_single_scalar(live[:], inv[:], 0.5,
                                                   Alu.is_ge)
                    w1p = small.tile([128, EP], fp32, tag="w1p", name="w1p")
                    nc.vector.tensor_single_scalar(w1p[:], wm[:], 1.0, Alu.add)
                    w2p = small.tile([128, EP], fp32, tag="w2p", name="w2p")
                    nc.vector.tensor_tensor(w2p[:], w1p[:], live[:], Alu.mult)
                    wfin = small.tile([128, EP], fp32, tag="wfin", name="wfin")
                    nc.vector.tensor_single_scalar(wfin[:], w2p[:], -1.0,
                                                   Alu.add)
                    oh = flow.tile([128, EP, V], bf16, tag="oh", name="oh")
                    nc.vector.tensor_tensor(
                        oh[:],
                        iota_t[:].unsqueeze(1).broadcast_to([128, EP, V]),
                        wfin[:].unsqueeze(2).broadcast_to([128, EP, V]),
                        Alu.is_equal)
                    nc.sync.dma_start(oute[m],
                                      oh[:].rearrange("p e v -> p (e v)"))

                accs = {(m, c): ps.tile([128, B], fp32, tag=f"p4_{m}_{c}",
                                        name=f"p4_{m}_{c}")
                        for m in range(MT) for c in range(NCH)}

                # open every accumulation group with the rank-1 b2 update
                # (depends only on constants -> fills the pre-mm2 stall)
                for m in range(MT):
                    for c in range(NCH):
                        nc.tensor.matmul(accs[(m, c)][:, :CC], one_t[:],
                                         b2_t[:, c * CC:(c + 1) * CC],
                                         start=True, stop=False)

                def visit_tiles(oi):
                    g, s = order[oi]
                    j = HM * s + g
                    if oi < NPRE:
                        w2rt, w2lt = w2pre[oi]
                    else:
                        w2rt = w2late.tile([128, CE], fp32r, tag="w2qr",
                                           name=f"w2qr{oi}")
                        nc.sync.dma_start(w2rt[:], w2r[:, oi, :])
                        w2lt = w2late.tile([128, CE], bf16, tag="w2ql",
                                           name=f"w2ql{oi}")
                        nc.sync.dma_start(w2lt[:], w2l[:, oi, :])
                    ht = hthp.tile([128, B], fp32r, tag="hth", name=f"hth{j}")
                    nc.sync.dma_start(ht[:], ag_out[g][s])
                    hb = hbp.tile([128, B], bf16, tag="hb", name=f"hb{j}")
                    nc.vector.tensor_copy(hb[:], ht[:])
                    return ht, hb, w2rt, w2lt

                def tile_mms(m, ht, hb, w2rt, w2lt, last):
                    lr = ht[:, m * 128:(m + 1) * 128]
                    lb = hb[:, m * 128:(m + 1) * 128]
                    for c in range(NCH):
                        nc.tensor.matmul(accs[(m, c)][:, :CC], lr,
                                         w2rt[:, c * CC:(c + 1) * CC],
                                         start=False, stop=False)
                    for c in range(NCH):
                        nc.tensor.matmul(accs[(m, c)][:, :CC], lb,
                                         w2lt[:, c * CC:(c + 1) * CC],
                                         start=False, stop=last)

                # groups g=0..2: j outer, m inner
                for oi in range(3 * NCORES):
                    ht, hb, w2rt, w2lt = visit_tiles(oi)
                    for m in range(MT):
                        tile_mms(m, ht, hb, w2rt, w2lt, last=False)
                # group g=3: m outer, j inner -> acc(m) closes early; its
                # epilogue + flow overlap the remaining batch tiles' matmuls
                g3 = [visit_tiles(oi) for oi in range(3 * NCORES, 4 * NCORES)]
                for m in range(MT):
                    for vi, (ht, hb, w2rt, w2lt) in enumerate(g3):
                        tile_mms(m, ht, hb, w2rt, w2lt,
                                 last=(vi == NCORES - 1))
                    for c in range(NCH):
                        mm2_epilogue(accs[(m, c)][:, :CC], c, m)
                    flow_out(m)

    nc.compile()
    return nc


def _split_bf16(a):
    hi = a.astype(BF16)
    lo = (a - hi.astype(np.float32)).astype(BF16)
    return hi, lo


def kernel(inputs, mask, W1, b1, W2, b2):
    from concourse.bass_utils import run_bass_kernel_spmd

    if "nc" not in _cache:
        _cache["nc"] = _build()
    nc = _cache["nc"]

    inputs = np.asarray(inputs, np.float32)
    mask = np.asarray(mask, np.float32)
    W1 = np.asarray(W1, np.float32)
    b1 = np.asarray(b1, np.float32)
    W2 = np.asarray(W2, np.float32)
    b2 = np.asarray(b2, np.float32)

    masked = inputs * mask[None, :, :]                    # [B, L, V]
    x_odd = masked[:, 1::2, :].reshape(B, (L // 2) * V)   # [512, 2944]
    xt_np = np.ascontiguousarray(
        x_odd.T.reshape(KT1, 128, B).transpose(1, 0, 2)).astype(BF16)
    W1_odd = W1.reshape(L, V, H)[1::2].reshape((L // 2) * V, H)

    in_maps = []
    for k in range(NCORES):
        w1s = W1_odd[:, k * HS:(k + 1) * HS]
        w1hi, w1lo = _split_bf16(w1s)
        # odd-position net columns are multiplied by (1-mask)=0 downstream:
        # only the 736 even-position columns of this core's W2 slice matter
        w2sl = W2[:, k * CW:(k + 1) * CW].reshape(H, PS, 2 * V)[:, 0::2, :]
        w2sl = w2sl.reshape(H, CE)
        w2r_np = _trunc11(w2sl)
        w2l_np = (w2sl - w2r_np).astype(BF16)
        # reorder contraction tiles into mm2 visit order: j = HM*s + g
        jorder = [HM * s + g for g in range(HM) for s in range(NCORES)]
        w2r_v = w2r_np.reshape(KT2, 128, CE)[jorder].transpose(1, 0, 2)
        w2l_v = w2l_np.reshape(KT2, 128, CE)[jorder].transpose(1, 0, 2)
        b2s = b2[k * CW:(k + 1) * CW].reshape(PS, 2 * V)[0::2].reshape(CE)
        cols = slice(32 * k, 32 * k + 32, 2)
        inpe_np = inputs[:, cols, :].reshape(MT, 128, EP * V)
        in_maps.append({
            "xt": xt_np,
            "w1h": np.ascontiguousarray(
                w1hi.reshape(KT1, 128, HS).transpose(1, 0, 2)),
            "w1l": np.ascontiguousarray(
                w1lo.reshape(KT1, 128, HS).transpose(1, 0, 2)),
            "b1s": np.ascontiguousarray(b1[k * HS:(k + 1) * HS].reshape(-1, 128)),
            "w2r": np.ascontiguousarray(w2r_v),
            "w2l": np.ascontiguousarray(w2l_v),
            "b2row": np.ascontiguousarray(_trunc11(b2s).reshape(1, CE)),
            "inpe": np.ascontiguousarray(inpe_np.astype(BF16)),
        })

    res = run_bass_kernel_spmd(nc, in_maps, core_ids=list(range(NCORES)))
    _cache["last_result"] = res

    out = np.empty((B, L, V), np.float32)
    out[:, 1::2, :] = masked[:, 1::2, :]
    for k in range(NCORES):
        oe = res.results[k]["oute"].astype(np.float32).reshape(MT, 128, EP, V)
        out[:, 32 * k:32 * k + 32:2, :] = oe.reshape(B, EP, V)
    return out


# revision 9
# speedup vs baseline: 1.2451x; 1.0404x over previous
"""DiscreteBipartiteFlow forward on 8 Trainium2 NeuronCores.

Math (forward pass only):
  masked = mask * inputs                      (mask = 1 at odd l, 0 at even l)
  h   = relu(masked.reshape(B, L*V) @ W1 + b1)
  net = (h @ W2 + b2).reshape(B, L, 2V)
  loc, scale = argmax one-hots of net[..., :V], net[..., V:]
  out[odd l]  = inputs
  out[even l] = onehot((inv(scale) * ((tok - loc) mod V)) mod V), or 0 if scale==0

st_one_hot_argmax's forward value is exactly the hard one-hot, so the
post-MLP flow is pure index arithmetic mod 23.

Sharding (8 cores):
  mm1: tensor-parallel over hidden; core k computes hT[512k:512k+512, :]
       from the odd-position one-hot rows (2944 of 5888 W1 rows).
       2 bf16 passes (W1 hi/lo; x one-hot is bf16-exact), fp32 PSUM,
       relu+bias via ScalarE activation rounding to fp32r output.
  all-gather: h as fp32r (4 B/elem), one chunk per local 128-row tile,
       pipelined; a zero-dep dummy collective at t=0 absorbs the ~60-80us
       ncfw setup latency.
  mm2: tensor-parallel over output columns (736 even-position cols/core),
       single sweep over 32 contraction tiles grouped by gather chunk.
       TWO passes, both 1 cycle/row:
         pass1: fp32r  h_r x W2_r     (11-bit-mantissa operands)
         pass2: bf16   h_hi x W2_lo   (W2_lo = W2 - W2_r; h_hi cast on the
                otherwise-idle VectorE)
       W2 is fully SBUF-resident: half preloaded under mm1, half loaded
       into the space freed by the mm1 operands during the pre-mm2 stall,
       so the gather chunks never contend with W2 streaming. b2 is folded
       in as a rank-1 matmul per accumulator at the START of the group.
       The last chunk group runs m-outer so each batch-tile's accumulators
       close early; its epilogue + flow overlap the remaining matmuls.
  epilogue: argmax via is_ge + max-of-(BIG-iota); modular flow per batch
       tile; inverse mod 23 via one-hot table lookup.
"""

import numpy as np
import ml_dtypes

B, L, V = 512, 256, 23
H = 4096
NCORES = 8
HS = H // NCORES          # 512  hidden shard
HM = HS // 128            # 4    local hidden tiles
PS = L // NCORES          # 32   positions per core
EP = PS // 2              # 16   even positions per core
CW = PS * 2 * V           # 1472 net columns per core (incl. unused odd)
CE = EP * 2 * V           # 736  even-position net columns (the used ones)
NCH = 2                   # column chunks for mm2
CC = CE // NCH            # 368  columns per chunk (8 even positions)
KT1 = (L // 2) * V // 128  # 23 contraction tiles for mm1
KT2 = H // 128            # 32 contraction tiles for mm2
MT = B // 128             # 4 batch tiles
NPRE = 16                 # mm2 tile visits with W2 preloaded under mm1

BIG = 64.0
MAGIC = 12582912.0        # 1.5 * 2^23: float32 round-to-int domain
BF16 = ml_dtypes.bfloat16

_cache = {}


def _trunc11(a):
    """Round fp32 to 11 explicit mantissa bits (RNE) == TRN2 fp32r."""
    a = np.asarray(a, np.float32)
    m, e = np.frexp(a)
    m2 = np.round(m * 4096.0) / 4096.0
    return np.ldexp(m2, e).astype(np.float32)


def _build():
    import concourse.mybir as mybir
    import concourse.tile as tile
    from concourse import bacc

    fp32 = mybir.dt.float32
    fp32r = mybir.dt.float32r
    bf16 = mybir.dt.bfloat16
    Alu = mybir.AluOpType
    Act = mybir.ActivationFunctionType

    nc = bacc.Bacc("TRN2", target_bir_lowering=False, debug=False,
                   num_devices=NCORES)

    # ---- per-core inputs ----
    xt = nc.dram_tensor("xt", [128, KT1, B], bf16, kind="ExternalInput")
    w1h = nc.dram_tensor("w1h", [128, KT1, HS], bf16, kind="ExternalInput")
    w1l = nc.dram_tensor("w1l", [128, KT1, HS], bf16, kind="ExternalInput")
    b1s = nc.dram_tensor("b1s", [HM, 128], fp32, kind="ExternalInput")
    # w2r/w2l are laid out in mm2 VISIT order on the host: [128, visit, CE]
    w2r = nc.dram_tensor("w2r", [128, KT2, CE], fp32r, kind="ExternalInput")
    w2l = nc.dram_tensor("w2l", [128, KT2, CE], bf16, kind="ExternalInput")
    b2row = nc.dram_tensor("b2row", [1, CE], fp32r, kind="ExternalInput")
    inpe = nc.dram_tensor("inpe", [MT, 128, EP * V], bf16, kind="ExternalInput")
    oute = nc.dram_tensor("oute", [MT, 128, EP * V], bf16,
                          kind="ExternalOutput")

    # ---- constants (baked into the NEFF) ----
    iota_np = np.arange(V, dtype=np.float32)[None, :].repeat(128, 0)
    inv_np = np.array([0] + [pow(a, -1, V) for a in range(1, V)],
                      dtype=np.float32)[None, :].repeat(128, 0).astype(BF16)
    c_iota = nc.inline_tensor(np.ascontiguousarray(iota_np), name="c_iota")
    c_iob = nc.inline_tensor(np.ascontiguousarray(iota_np.astype(BF16)),
                             name="c_iob")
    c_bi = nc.inline_tensor(np.ascontiguousarray((BIG - iota_np).astype(BF16)),
                            name="c_bi")
    c_inv = nc.inline_tensor(np.ascontiguousarray(inv_np), name="c_inv")
    c_one = nc.inline_tensor(np.ones((1, 128), np.float32), name="c_one")

    # mm2 visit order: contraction tiles grouped by gather chunk
    order = [(g, s) for g in range(HM) for s in range(NCORES)]

    with tile.TileContext(nc) as tc:
        with (
            tc.tile_pool(name="persist", bufs=1) as persist,
            tc.tile_pool(name="hth", bufs=8) as hthp,
            tc.tile_pool(name="hb", bufs=8) as hbp,
            tc.tile_pool(name="ep", bufs=2) as ep,
            tc.tile_pool(name="flow", bufs=1) as flow,
            tc.tile_pool(name="small", bufs=2) as small,
            tc.tile_pool(name="ps", bufs=1, space="PSUM") as ps,
            tc.tile_pool(name="dram", bufs=1, space="DRAM") as dram,
        ):
            # ---------- dummy collective: absorbs ncfw setup latency --------
            warm_in = dram.tile([1, 16], fp32, tag="warm_in")
            warm_out = dram.tile([NCORES, 16], fp32, tag="warm_out",
                                 addr_space="Shared")
            nc.gpsimd.collective_compute(
                "AllGather", Alu.bypass,
                replica_groups=[list(range(NCORES))],
                ins=[warm_in.opt()], outs=[warm_out.opt()],
            )

            # ---------- constants to SBUF ----------
            iota_t = persist.tile([128, V], fp32, tag="iota")
            nc.sync.dma_start(iota_t[:], c_iota[:])
            iob_t = persist.tile([128, V], bf16, tag="iob")
            nc.sync.dma_start(iob_t[:], c_iob[:])
            cbi_t = persist.tile([128, V], bf16, tag="cbi")
            nc.sync.dma_start(cbi_t[:], c_bi[:])
            inv_t = persist.tile([128, V], bf16, tag="inv")
            nc.sync.dma_start(inv_t[:], c_inv[:])
            one_t = persist.tile([1, 128], fp32r, tag="one")
            nc.sync.dma_start(one_t[:], c_one[:].bitcast(fp32r))
            b1_t = []
            for m in range(HM):
                t = persist.tile([128, 1], fp32, tag=f"b1{m}")
                nc.sync.dma_start(t[:], b1s[m].unsqueeze(1))
                b1_t.append(t)
            b2_t = persist.tile([1, CE], fp32r, tag="b2")
            nc.sync.dma_start(b2_t[:], b2row[:])

            # ---------- per-chunk collective buffers ----------
            ag_in = [dram.tile([128, B], fp32r, tag=f"ag_in{m}",
                               name=f"ag_in{m}") for m in range(HM)]
            ag_out = [dram.tile([NCORES, 128, B], fp32r, tag=f"ag_out{m}",
                                name=f"ag_out{m}", addr_space="Shared")
                      for m in range(HM)]

            # ---------- W2 preload for the first NPRE visits ----------------
            w2pre = {}
            hr_t = [None] * HM

            with tc.tile_pool(name="mm1ops", bufs=1) as mm1p:
                # mm1 operand loads in a few big descriptors, chunked so the
                # first matmuls can start before the whole stream lands
                # (per-descriptor sync issue costs ~0.5us -- fewer is faster)
                xtt = mm1p.tile([128, KT1, B], bf16, tag="xtt", name="xtt")
                w1ht = mm1p.tile([128, KT1, HS], bf16, tag="w1ht", name="w1ht")
                w1lt = mm1p.tile([128, KT1, HS], bf16, tag="w1lt", name="w1lt")
                kr = [(0, 6), (6, 12), (12, 18), (18, KT1)]
                for k0, k1 in kr:
                    nc.sync.dma_start(xtt[:, k0:k1, :], xt[:, k0:k1, :])
                    nc.sync.dma_start(w1ht[:, k0:k1, :], w1h[:, k0:k1, :])
                for k0, k1 in [(0, 12), (12, KT1)]:
                    nc.sync.dma_start(w1lt[:, k0:k1, :], w1l[:, k0:k1, :])
                xt_t = [xtt[:, k, :] for k in range(KT1)]
                w1h_t = [w1ht[:, k, :] for k in range(KT1)]
                w1l_t = [w1lt[:, k, :] for k in range(KT1)]

                w2preR = persist.tile([128, NPRE, CE], fp32r, tag="w2preR",
                                      name="w2preR")
                nc.sync.dma_start(w2preR[:], w2r[:, 0:NPRE, :])
                w2preL = persist.tile([128, NPRE, CE], bf16, tag="w2preL",
                                      name="w2preL")
                nc.sync.dma_start(w2preL[:], w2l[:, 0:NPRE, :])
                for oi in range(NPRE):
                    w2pre[oi] = (w2preR[:, oi, :], w2preL[:, oi, :])

                # ------- phase 1: mm1 -> relu -> fp32r round -> gather ------
                for m in range(HM):
                    acc = ps.tile([128, B], fp32, tag=f"p4_{m}_0",
                                  name=f"ps1_{m}")
                    for k in range(KT1):
                        nc.tensor.matmul(acc[:],
                                         w1h_t[k][:, m * 128:(m + 1) * 128],
                                         xt_t[k], start=(k == 0), stop=False)
                    for k in range(KT1):
                        nc.tensor.matmul(acc[:],
                                         w1l_t[k][:, m * 128:(m + 1) * 128],
                                         xt_t[k], start=False,
                                         stop=(k == KT1 - 1))
                    hr = persist.tile([128, B], fp32r, tag=f"hr{m}",
                                      name=f"hr{m}")
                    nc.scalar.activation(hr[:], acc[:], Act.Relu, bias=b1_t[m],
                                         scale=1.0)
                    hr_t[m] = hr
                    nc.sync.dma_start(ag_in[m][:], hr[:])
                    nc.gpsimd.collective_compute(
                        "AllGather", Alu.bypass,
                        replica_groups=[list(range(NCORES))],
                        ins=[ag_in[m].opt()], outs=[ag_out[m].opt()],
                    )

                # token index per batch tile (dep-free, runs during mm1)
                tok_t = persist.tile([128, MT, EP], fp32, tag="tok")
                for m in range(MT):
                    it = ep.tile([128, EP * V], bf16, tag="inpe")
                    nc.sync.dma_start(it[:], inpe[m])
                    tmp = ep.tile([128, EP, V], bf16, tag="tokmul")
                    nc.vector.tensor_tensor(
                        tmp[:], it[:].rearrange("p (e v) -> p e v", v=V),
                        iob_t[:].unsqueeze(1).broadcast_to([128, EP, V]),
                        Alu.mult)
                    nc.vector.tensor_reduce(tok_t[:, m], tmp[:],
                                            axis=mybir.AxisListType.X,
                                            op=Alu.add)

            # mm1 operand pool closed: its SBUF is reused for the second half
            # of W2, DMA'd during the pre-mm2 stall (no gather contention).
            # second half of W2 loads via a rotating pool: each load waits
            # (WAR) for the allocation 8 slots earlier to be fully read. A
            # pre-generation of gate tiles (tiny DMA from the chunk-0 gather
            # output, then a tiny vector read) makes the first 8 real loads
            # wait for chunk 0, keeping its gather wire contention-free.
            with tc.tile_pool(name="w2late", bufs=8) as w2late:
                idx_t = persist.tile([128, MT, EP, 2], fp32, tag="idx")

                # ---------- phase 2: mm2 ----------------------------------
                def mm2_epilogue(acc, nch, m):
                    # acc: [128, 368] = 8 even positions x (loc|scale) x 23
                    ng = acc.rearrange("p (i s v) -> p i s v", s=2, v=V)
                    gmax = ep.tile([128, 8, 2], fp32, tag="gmax", name="gmax")
                    nc.vector.tensor_reduce(gmax[:], ng,
                                            axis=mybir.AxisListType.X,
                                            op=Alu.max)
                    eq = ep.tile([128, 8, 2, V], bf16, tag="eq", name="eq")
                    nc.vector.tensor_tensor(
                        eq[:], ng,
                        gmax[:].unsqueeze(3).broadcast_to([128, 8, 2, V]),
                        Alu.is_ge)
                    mt = ep.tile([128, 8, 2, V], bf16, tag="mt", name="mt")
                    nc.vector.tensor_tensor(
                        mt[:], eq[:],
                        cbi_t[:].unsqueeze(1).unsqueeze(1).broadcast_to(
                            [128, 8, 2, V]), Alu.mult)
                    tmax = ep.tile([128, 8, 2], bf16, tag="tmax", name="tmax")
                    nc.vector.tensor_reduce(tmax[:], mt[:],
                                            axis=mybir.AxisListType.X,
                                            op=Alu.max)
                    nc.vector.tensor_scalar(
                        idx_t[:, m, nch * 8:(nch + 1) * 8, :],
                        tmax[:], -1.0, BIG, Alu.mult, Alu.add)

                def mod23(dst_tag, src):
                    d = small.tile([128, EP], fp32, tag=dst_tag + "_d",
                                   name=dst_tag + "_d")
                    nc.vector.tensor_scalar(d[:], src[:], 1.0 / 23.0, -0.49,
                                            Alu.mult, Alu.add)
                    q = small.tile([128, EP], fp32, tag=dst_tag + "_q",
                                   name=dst_tag + "_q")
                    nc.vector.tensor_scalar(q[:], d[:], MAGIC, MAGIC,
                                            Alu.add, Alu.subtract)
                    r = small.tile([128, EP], fp32, tag=dst_tag + "_r",
                                   name=dst_tag + "_r")
                    nc.vector.scalar_tensor_tensor(r[:], q[:], -23.0, src[:],
                                                   Alu.mult, Alu.add)
                    return r

                def flow_out(m):
                    """argmax indices -> modular flow -> one-hot -> DRAM."""
                    loc = idx_t[:, m, :, 0]
                    scl = idx_t[:, m, :, 1]
                    tok = tok_t[:, m]
                    u0 = small.tile([128, EP], fp32, tag="u0", name="u0")
                    nc.vector.scalar_tensor_tensor(u0[:], tok, 23.0, loc,
                                                   Alu.add, Alu.subtract)
                    geu = small.tile([128, EP], fp32, tag="geu", name="geu")
                    nc.vector.tensor_single_scalar(geu[:], u0[:], 23.0,
                                                   Alu.is_ge)
                    u = small.tile([128, EP], fp32, tag="u", name="u")
                    nc.vector.scalar_tensor_tensor(u[:], geu[:], -23.0, u0[:],
                                                   Alu.mult, Alu.add)
                    # inv = inv_table[scl] via one-hot dot
                    eqi = flow.tile([128, EP, V], bf16, tag="eqi", name="eqi")
                    nc.vector.tensor_tensor(
                        eqi[:],
                        iota_t[:].unsqueeze(1).broadcast_to([128, EP, V]),
                        scl.unsqueeze(2).broadcast_to([128, EP, V]),
                        Alu.is_equal)
                    isel = flow.tile([128, EP, V], bf16, tag="isel",
                                     name="isel")
                    nc.vector.tensor_tensor(
                        isel[:], eqi[:],
                        inv_t[:].unsqueeze(1).broadcast_to([128, EP, V]),
                        Alu.mult)
                    inv = small.tile([128, EP], fp32, tag="invv", name="invv")
                    nc.vector.tensor_reduce(inv[:], isel[:],
                                            axis=mybir.AxisListType.X,
                                            op=Alu.add)
                    wprod = small.tile([128, EP], fp32, tag="wprod",
                                       name="wprod")
                    nc.vector.tensor_tensor(wprod[:], inv[:], u[:], Alu.mult)
                    wm = mod23("wm", wprod)
                    live = small.tile([128, EP], fp32, tag="live", name="live")
                    nc.vector.tensor_single_scalar(live[:], inv[:], 0.5,
                                                   Alu.is_ge)
                    w1p = small.tile([128, EP], fp32, tag="w1p", name="w1p")
                    nc.vector.tensor_single_scalar(w1p[:], wm[:], 1.0, Alu.add)
                    w2p = small.tile([128, EP], fp32, tag="w2p", name="w2p")
                    nc.vector.tensor_tensor(w2p[:], w1p[:], live[:], Alu.mult)
                    wfin = small.tile([128, EP], fp32, tag="wfin", name="wfin")
                    nc.vector.tensor_single_scalar(wfin[:], w2p[:], -1.0,
                                                   Alu.add)
                    oh = flow.tile([128, EP, V], bf16, tag="oh", name="oh")
                    nc.vector.tensor_tensor(
                        oh[:],
                        iota_t[:].unsqueeze(1).broadcast_to([128, EP, V]),
                        wfin[:].unsqueeze(2).broadcast_to([128, EP, V]),
                        Alu.is_equal)
                    nc.sync.dma_start(oute[m],
                                      oh[:].rearrange("p e v -> p (e v)"))

                accs = {(m, c): ps.tile([128, B], fp32, tag=f"p4_{m}_{c}",
                                        name=f"p4_{m}_{c}")
                        for m in range(MT) for c in range(NCH)}

                # open every accumulation group with the rank-1 b2 update
                # (depends only on constants -> fills the pre-mm2 stall)
                for m in range(MT):
                    for c in range(NCH):
                        nc.tensor.matmul(accs[(m, c)][:, :CC], one_t[:],
                                         b2_t[:, c * CC:(c + 1) * CC],
                                         start=True, stop=False)

                def visit_tiles(oi):
                    g, s = order[oi]
                    j = HM * s + g
                    if oi < NPRE:
                        w2rt, w2lt = w2pre[oi]
                    else:
                        w2rt = w2late.tile([128, CE], fp32r, tag="w2qr",
                                           name=f"w2qr{oi}")
                        nc.sync.dma_start(w2rt[:], w2r[:, oi, :])
                        w2lt = w2late.tile([128, CE], bf16, tag="w2ql",
                                           name=f"w2ql{oi}")
                        nc.sync.dma_start(w2lt[:], w2l[:, oi, :])
                    ht = hthp.tile([128, B], fp32r, tag="hth", name=f"hth{j}")
                    nc.sync.dma_start(ht[:], ag_out[g][s])
                    hb = hbp.tile([128, B], bf16, tag="hb", name=f"hb{j}")
                    nc.vector.tensor_copy(hb[:], ht[:])
                    return ht, hb, w2rt, w2lt

                def tile_mms(m, ht, hb, w2rt, w2lt, last):
                    lr = ht[:, m * 128:(m + 1) * 128]
                    lb = hb[:, m * 128:(m + 1) * 128]
                    for c in range(NCH):
                        nc.tensor.matmul(accs[(m, c)][:, :CC], lr,
                                         w2rt[:, c * CC:(c + 1) * CC],
                                         start=False, stop=False)
                    for c in range(NCH):
                        nc.tensor.matmul(accs[(m, c)][:, :CC], lb,
                                         w2lt[:, c * CC:(c + 1) * CC],
                                         start=False, stop=last)

                # group g=0: j outer, m inner (emitted first so its h-tile
                # reads win the DMA queue race against the gated W2 loads)
                for oi in range(NCORES):
                    ht, hb, w2rt, w2lt = visit_tiles(oi)
                    for m in range(MT):
                        tile_mms(m, ht, hb, w2rt, w2lt, last=False)
                # gate pre-generation: holds the first 8 late-W2 loads (WAR
                # through the rotating pool) until the chunk-0 gather lands
                for i in range(8):
                    gr = w2late.tile([128, CE], fp32r, tag="w2qr",
                                     name=f"gr{i}")
                    nc.sync.dma_start(gr[0:1, 0:1], ag_out[0][0, 0:1, 0:1])
                    sr = small.tile([1, 1], fp32, tag="gsr", name="gsr")
                    nc.vector.tensor_copy(sr[:], gr[0:1, 0:1].bitcast(fp32))
                    gl = w2late.tile([128, CE], bf16, tag="w2ql",
                                     name=f"gl{i}")
                    nc.sync.dma_start(
                        gl[0:1, 0:1],
                        ag_out[0][:].bitcast(bf16)[0, 0:1, 0:1])
                    sl = small.tile([1, 1], fp32, tag="gsl", name="gsl")
                    nc.vector.tensor_copy(sl[:], gl[0:1, 0:1])
                # groups g=1..2: j outer, m inner
                for oi in range(NCORES, 3 * NCORES):
                    ht, hb, w2rt, w2lt = visit_tiles(oi)
                    for m in range(MT):
                        tile_mms(m, ht, hb, w2rt, w2lt, last=False)
                # group g=3: m outer, j inner -> acc(m) closes early; its
                # epilogue + flow overlap the remaining batch tiles' matmuls
                g3 = [visit_tiles(oi) for oi in range(3 * NCORES, 4 * NCORES)]
                for m in range(MT):
                    for vi, (ht, hb, w2rt, w2lt) in enumerate(g3):
                        tile_mms(m, ht, hb, w2rt, w2lt,
                                 last=(vi == NCORES - 1))
                    for c in range(NCH):
                        mm2_epilogue(accs[(m, c)][:, :CC], c, m)
                    flow_out(m)

    nc.compile()
    return nc


def _split_bf16(a):
    hi = a.astype(BF16)
    lo = (a - hi.astype(np.float32)).astype(BF16)
    return hi, lo


def kernel(inputs, mask, W1, b1, W2, b2):
    from concourse.bass_utils import run_bass_kernel_spmd

    if "nc" not in _cache:
        _cache["nc"] = _build()
    nc = _cache["nc"]

    inputs = np.asarray(inputs, np.float32)
    mask = np.asarray(mask, np.float32)
    W1 = np.asarray(W1, np.float32)
    b1 = np.asarray(b1, np.float32)
    W2 = np.asarray(W2, np.float32)
    b2 = np.asarray(b2, np.float32)

    masked = inputs * mask[None, :, :]                    # [B, L, V]
    x_odd = masked[:, 1::2, :].reshape(B, (L // 2) * V)   # [512, 2944]
    xt_np = np.ascontiguousarray(
        x_odd.T.reshape(KT1, 128, B).transpose(1, 0, 2)).astype(BF16)
    W1_odd = W1.reshape(L, V, H)[1::2].reshape((L // 2) * V, H)

    in_maps = []
    for k in range(NCORES):
        w1s = W1_odd[:, k * HS:(k + 1) * HS]
        w1hi, w1lo = _split_bf16(w1s)
        # odd-position net columns are multiplied by (1-mask)=0 downstream:
        # only the 736 even-position columns of this core's W2 slice matter
        w2sl = W2[:, k * CW:(k + 1) * CW].reshape(H, PS, 2 * V)[:, 0::2, :]
        w2sl = w2sl.reshape(H, CE)
        w2r_np = _trunc11(w2sl)
        w2l_np = (w2sl - w2r_np).astype(BF16)
        # reorder contraction tiles into mm2 visit order: j = HM*s + g
        jorder = [HM * s + g for g in range(HM) for s in range(NCORES)]
        w2r_v = w2r_np.reshape(KT2, 128, CE)[jorder].transpose(1, 0, 2)
        w2l_v = w2l_np.reshape(KT2, 128, CE)[jorder].transpose(1, 0, 2)
        b2s = b2[k * CW:(k + 1) * CW].reshape(PS, 2 * V)[0::2].reshape(CE)
        cols = slice(32 * k, 32 * k + 32, 2)
        inpe_np = inputs[:, cols, :].reshape(MT, 128, EP * V)
        in_maps.append({
            "xt": xt_np,
            "w1h": np.ascontiguousarray(
                w1hi.reshape(KT1, 128, HS).transpose(1, 0, 2)),
            "w1l": np.ascontiguousarray(
                w1lo.reshape(KT1, 128, HS).transpose(1, 0, 2)),
            "b1s": np.ascontiguousarray(b1[k * HS:(k + 1) * HS].reshape(-1, 128)),
            "w2r": np.ascontiguousarray(w2r_v),
            "w2l": np.ascontiguousarray(w2l_v),
            "b2row": np.ascontiguousarray(_trunc11(b2s).reshape(1, CE)),
            "inpe": np.ascontiguousarray(inpe_np.astype(BF16)),
        })

    res = run_bass_kernel_spmd(nc, in_maps, core_ids=list(range(NCORES)))
    _cache["last_result"] = res

    out = np.empty((B, L, V), np.float32)
    out[:, 1::2, :] = masked[:, 1::2, :]
    for k in range(NCORES):
        oe = res.results[k]["oute"].astype(np.float32).reshape(MT, 128, EP, V)
        out[:, 32 * k:32 * k + 32:2, :] = oe.reshape(B, EP, V)
    return out
